# revision 6
# baseline (speedup 1.0000x reference)
"""Distributed Trainium2 (Bass/Tile) kernel for a llama-style attention layer.

Problem: B=4, S=1024, D=4096, H=32 heads, HD=128; fp32 in/out; start_pos=0,
cache starts zeroed (the cache update is a pure overwrite, so the layer is
exactly: QKV proj -> RoPE(q,k) -> softmax((q k^T)/sqrt(HD) + mask) v -> Wo).

Sharding (8 NeuronCores; tensor-parallel over heads, then all-to-all to
row-parallel for the output projection):
  phase 1: each core owns 4 heads; computes Q.T/K.T [hd, r] for its heads
           over all r = b*S + s rows (wq/wk column shards resident in SBUF,
           x streamed), applies RoPE in [hd(partition), r] layout via a
           partition pair-swap (stream_shuffle) with host-precomputed
           cos/sin tables (score 1/sqrt(HD) folded into the Q tables),
           spills Q.T/K.T to DRAM scratch.
  phase 2: per batch: V pass (V [s, m], wv resident, x streamed again), then
           attention per head computed transposed (S.T[k,q] = K @ Q.T) so no
           on-chip transposes are needed; softmax row-sums via a ones-vector
           matmul; PV as out.T[hd, q] with N=512 matmuls; normalized out.T
           tiles land in the all-to-all input buffer.
  phase 3: one AllToAll (8 MB/core) turns head-sharding into row-sharding;
           each core computes its 512 rows of the final projection against
           the full (streamed) wo; host concatenates the row shards.

All matmuls run in float32r (TF32-like; full PE rate at N=512).
"""

import math

import numpy as np

import concourse.bass as bass
import concourse.mybir as mybir
import concourse.tile as tile
from concourse import bacc
from concourse.bass_utils import run_bass_kernel_spmd

F32 = mybir.dt.float32
F32R = mybir.dt.float32r
BF16 = mybir.dt.bfloat16
AF = mybir.ActivationFunctionType

B, S, D, H = 4, 1024, 4096, 32
HD = 128
NCORES = 8
HL = H // NCORES   # 4 local heads
M = HL * HD        # 512 local head columns
R = B * S          # 4096 rows
RPC = R // NCORES  # 512 rows per core in phase 3
P = 128
DC = D // P        # 32 contraction chunks

_BUILD_CACHE = {}


def _build():
    if "nc" in _BUILD_CACHE:
        return _BUILD_CACHE["nc"]
    nc = bacc.Bacc(None, target_bir_lowering=False)

    xT = nc.declare_dram_parameter("xT", [D, R], F32, isOutput=False)
    wqT = nc.declare_dram_parameter("wqT", [D, M], F32, isOutput=False)
    wkT = nc.declare_dram_parameter("wkT", [D, M], F32, isOutput=False)
    wvT = nc.declare_dram_parameter("wvT", [D, M], F32, isOutput=False)
    woT = nc.declare_dram_parameter("woT", [D, D], F32, isOutput=False)
    cq = nc.declare_dram_parameter("cq", [P, S], F32, isOutput=False)
    sq = nc.declare_dram_parameter("sq", [P, S], F32, isOutput=False)
    ck = nc.declare_dram_parameter("ck", [P, S], F32, isOutput=False)
    sk = nc.declare_dram_parameter("sk", [P, S], F32, isOutput=False)
    maskT = nc.declare_dram_parameter("maskT", [S, S], F32, isOutput=False)
    out = nc.declare_dram_parameter("out", [RPC, D], F32, isOutput=True)

    pairmask = [p ^ 1 for p in range(32)]

    with tile.TileContext(nc) as tc:
        with (
            tc.tile_pool(name="const", bufs=1) as cpool,
            tc.tile_pool(name="dram", bufs=1, space="DRAM") as dram,
        ):
            # DRAM scratch: QK spill [2(q/k), B, HL, 128, S] and a2a buffers
            qkd = dram.tile([2, B, HL, P, S], F32R, tag="qkd")
            a2a_in = dram.tile([R, RPC], F32, tag="a2a_in")
            a2a_out = dram.tile([R, RPC], F32, tag="a2a_out")

            ones_f = cpool.tile([P, P], F32, tag="ones_f")
            nc.vector.memset(ones_f[:], 1.0)
            ones128 = cpool.tile([P, P], F32R, tag="ones128")
            nc.vector.tensor_copy(ones128[:], ones_f[:])

            # ---------------- phase 1: Q.T / K.T + rope -> spill ----------
            with (
                tc.tile_pool(name="p1w", bufs=1) as wpool,
                tc.tile_pool(name="p1rope", bufs=1) as rpool,
                tc.tile_pool(name="p1x", bufs=4) as xpool,
                tc.tile_pool(name="p1s", bufs=2) as spool,
                tc.tile_pool(name="p1ps", bufs=8, space="PSUM") as ps1,
            ):
                cq_sb = rpool.tile([P, S], F32, tag="cq")
                sq_sb = rpool.tile([P, S], F32, tag="sq")
                ck_sb = rpool.tile([P, S], F32, tag="ck")
                sk_sb = rpool.tile([P, S], F32, tag="sk")
                nc.sync.dma_start(cq_sb[:], cq[:])
                nc.sync.dma_start(sq_sb[:], sq[:])
                nc.sync.dma_start(ck_sb[:], ck[:])
                nc.sync.dma_start(sk_sb[:], sk[:])
                wq_sb = []
                wk_sb = []
                for dc in range(DC):
                    tq = wpool.tile([P, M], F32R, tag=f"wq{dc}")
                    tk = wpool.tile([P, M], F32R, tag=f"wk{dc}")
                    nc.gpsimd.dma_start(tq[:], wqT[dc * P:(dc + 1) * P, :])
                    nc.gpsimd.dma_start(tk[:], wkT[dc * P:(dc + 1) * P, :])
                    wq_sb.append(tq)
                    wk_sb.append(tk)

                for b in range(B):
                    for rt in range(2):
                        r0 = b * S + rt * 512
                        sl = slice(rt * 512, rt * 512 + 512)
                        q_ps = [ps1.tile([P, 512], F32, tag="mm", name=f"qps{m}") for m in range(HL)]
                        k_ps = [ps1.tile([P, 512], F32, tag="mm", name=f"kps{m}") for m in range(HL)]
                        for dc in range(DC):
                            xt = xpool.tile([P, 512], F32R, tag="xt")
                            nc.gpsimd.dma_start(
                                xt[:], xT[dc * P:(dc + 1) * P, r0:r0 + 512]
                            )
                            st = dc == 0
                            sp = dc == DC - 1
                            for m in range(HL):
                                nc.tensor.matmul(
                                    q_ps[m][:],
                                    wq_sb[dc][:, m * P:(m + 1) * P],
                                    xt[:],
                                    start=st, stop=sp,
                                )
                            for m in range(HL):
                                nc.tensor.matmul(
                                    k_ps[m][:],
                                    wk_sb[dc][:, m * P:(m + 1) * P],
                                    xt[:],
                                    start=st, stop=sp,
                                )
                        # rope: dst = src*c + pairswap(src)*s (sin sign folded)
                        for m in range(HL):
                            for qk, src, ctab, stab in (
                                (0, q_ps[m], cq_sb, sq_sb),
                                (1, k_ps[m], ck_sb, sk_sb),
                            ):
                                sh = spool.tile([P, 512], F32, tag="ropesh")
                                nc.vector.stream_shuffle(sh[:], src[:], pairmask)
                                t1 = spool.tile([P, 512], F32, tag="ropet1")
                                nc.vector.tensor_mul(t1[:], src[:], ctab[:, sl])
                                dst = spool.tile([P, 512], F32R, tag="ropedst")
                                nc.vector.tensor_mul(sh[:], sh[:], stab[:, sl])
                                nc.vector.tensor_add(dst[:], t1[:], sh[:])
                                nc.sync.dma_start(
                                    qkd[qk, b, m, :, sl], dst[:]
                                )

            # ---------------- phase 2: V pass + attention ----------------
            with (
                tc.tile_pool(name="p2w", bufs=1) as vwpool,
                tc.tile_pool(name="p2v", bufs=2) as vpool,
                tc.tile_pool(name="p2x", bufs=4) as xpool2,
                tc.tile_pool(name="p2qk", bufs=2) as qkpool,
                tc.tile_pool(name="p2ep", bufs=10) as eppool,
                tc.tile_pool(name="p2s", bufs=2) as apool,
                tc.tile_pool(name="p2m", bufs=1) as mpool,
                tc.tile_pool(name="p2ps", bufs=1, space="PSUM") as ps2,
            ):
                mk = {}
                for kt in range(8):
                    for qh in range(2):
                        t = mpool.tile([P, 512], BF16, tag=f"mk{kt}_{qh}")
                        nc.gpsimd.dma_start(
                            t[:],
                            maskT[kt * P:(kt + 1) * P, qh * 512:(qh + 1) * 512],
                        )
                        mk[(kt, qh)] = t
                wv_sb = []
                for dc in range(DC):
                    tv = vwpool.tile([P, M], F32R, tag=f"wv{dc}")
                    nc.gpsimd.dma_start(tv[:], wvT[dc * P:(dc + 1) * P, :])
                    wv_sb.append(tv)

                for b in range(B):
                    V = [vpool.tile([P, M], F32R, tag=f"V{i}", name=f"V{i}") for i in range(8)]
                    for rt in range(2):
                        r0 = b * S + rt * 512
                        v_ps = [
                            ps2.tile([P, M], F32, tag="vps", bufs=4, name=f"vps{i}")
                            for i in range(4)
                        ]
                        for dc in range(DC):
                            xt = xpool2.tile([P, 512], F32R, tag="xt")
                            nc.gpsimd.dma_start(
                                xt[:], xT[dc * P:(dc + 1) * P, r0:r0 + 512]
                            )
                            st = dc == 0
                            sp = dc == DC - 1
                            for sub in range(4):
                                nc.tensor.matmul(
                                    v_ps[sub][:],
                                    xt[:, sub * P:(sub + 1) * P],
                                    wv_sb[dc][:],
                                    start=st, stop=sp,
                                )
                        for sub in range(4):
                            nc.scalar.copy(V[rt * 4 + sub][:], v_ps[sub][:])

                    for h in range(HL):
                        QT = qkpool.tile([P, S], F32R, tag="QT")
                        KT = qkpool.tile([P, S], F32R, tag="KT")
                        nc.sync.dma_start(QT[:], qkd[0, b, h, :, :])
                        nc.sync.dma_start(KT[:], qkd[1, b, h, :, :])
                        for qh in range(2):
                            qsl = slice(qh * 512, qh * 512 + 512)
                            ep = []
                            for kt in range(8):
                                stile = ps2.tile([P, 512], F32, tag="st", bufs=2)
                                nc.tensor.matmul(
                                    stile[:],
                                    KT[:, kt * P:(kt + 1) * P],
                                    QT[:, qsl],
                                    start=True, stop=True,
                                )
                                nc.vector.tensor_add(
                                    stile[:], stile[:], mk[(kt, qh)][:]
                                )
                                e = eppool.tile([P, 512], F32R, tag="ep")
                                nc.scalar.activation(e[:], stile[:], AF.Exp)
                                ep.append(e)
                            # softmax denominator
                            s01 = apool.tile([P, 512], F32, tag="s01")
                            s23 = apool.tile([P, 512], F32, tag="s23")
                            s45 = apool.tile([P, 512], F32, tag="s45")
                            s67 = apool.tile([P, 512], F32, tag="s67")
                            nc.vector.tensor_add(s01[:], ep[0][:], ep[1][:])
                            nc.vector.tensor_add(s23[:], ep[2][:], ep[3][:])
                            nc.vector.tensor_add(s45[:], ep[4][:], ep[5][:])
                            nc.vector.tensor_add(s67[:], ep[6][:], ep[7][:])
                            nc.vector.tensor_add(s01[:], s01[:], s23[:])
                            nc.vector.tensor_add(s45[:], s45[:], s67[:])
                            accr = apool.tile([P, 512], F32R, tag="accr")
                            nc.vector.tensor_add(accr[:], s01[:], s45[:])
                            # rowsum broadcast to all partitions in one matmul
                            bcp = ps2.tile([P, 512], F32, tag="small", bufs=1)
                            nc.tensor.matmul(
                                bcp[:], ones128[:], accr[:], start=True, stop=True
                            )
                            bc_sb = apool.tile([P, 512], F32, tag="bcsb")
                            nc.vector.reciprocal(bc_sb[:], bcp[:])
                            # PV: out.T [hd, q]
                            ot = ps2.tile([P, 512], F32, tag="ot", bufs=1)
                            for kt in range(8):
                                nc.tensor.matmul(
                                    ot[:],
                                    V[kt][:, h * HD:(h + 1) * HD],
                                    ep[kt][:],
                                    start=(kt == 0), stop=(kt == 7),
                                )
                            otn = apool.tile([P, 512], F32, tag="otn")
                            nc.vector.tensor_mul(otn[:], ot[:], bc_sb[:])
                            j = b * 2 + qh
                            row0 = j * 512 + h * HD
                            nc.sync.dma_start(a2a_in[row0:row0 + HD, :], otn[:])

            # ---------------- phase 3: all-to-all + output proj -----------
            nc.gpsimd.collective_compute(
                "AllToAll",
                mybir.AluOpType.bypass,
                replica_groups=[list(range(NCORES))],
                ins=[a2a_in.opt()],
                outs=[a2a_out.opt()],
            )
            with (
                tc.tile_pool(name="p3a", bufs=1) as fpool,
                tc.tile_pool(name="p3w", bufs=4) as wspool,
                tc.tile_pool(name="p3o", bufs=4) as fopool,
                tc.tile_pool(name="p3ps", bufs=8, space="PSUM") as ps3,
            ):
                a2a_sb = []
                for ec in range(DC):
                    t = fpool.tile([P, RPC], F32R, tag=f"a2a{ec}")
                    nc.gpsimd.dma_start(t[:], a2a_out[ec * P:(ec + 1) * P, :])
                    a2a_sb.append(t)
                for dt_ in range(8):
                    f_ps = [ps3.tile([P, 512], F32, tag="fps", name=f"fps{i}") for i in range(4)]
                    for ec in range(DC):
                        wo_t = wspool.tile([P, 512], F32R, tag="wot")
                        nc.gpsimd.dma_start(
                            wo_t[:],
                            woT[ec * P:(ec + 1) * P, dt_ * 512:(dt_ + 1) * 512],
                        )
                        st = ec == 0
                        sp = ec == DC - 1
                        for qs in range(4):
                            nc.tensor.matmul(
                                f_ps[qs][:],
                                a2a_sb[ec][:, qs * P:(qs + 1) * P],
                                wo_t[:],
                                start=st, stop=sp,
                            )
                    for qs in range(4):
                        o_sb = fopool.tile([P, 512], F32, tag="osb")
                        nc.scalar.copy(o_sb[:], f_ps[qs][:])
                        nc.sync.dma_start(
                            out[qs * P:(qs + 1) * P, dt_ * 512:(dt_ + 1) * 512],
                            o_sb[:],
                        )

    nc.compile()
    _BUILD_CACHE["nc"] = nc
    return nc


def _prep_in_maps(x, wq, wk, wv, wo, freqs_cos, freqs_sin, mask):
    x = np.ascontiguousarray(np.asarray(x, dtype=np.float32))
    wq = np.asarray(wq, dtype=np.float32)
    wk = np.asarray(wk, dtype=np.float32)
    wv = np.asarray(wv, dtype=np.float32)
    wo = np.asarray(wo, dtype=np.float32)
    fc = np.asarray(freqs_cos, dtype=np.float32)  # [S, HD//2]
    fs = np.asarray(freqs_sin, dtype=np.float32)
    mask = np.asarray(mask, dtype=np.float32).reshape(S, S)

    xT = np.ascontiguousarray(x.reshape(R, D).T)          # [d, r]
    woT = np.ascontiguousarray(wo.T)                       # [e, dout]
    maskT = np.ascontiguousarray(mask.T)                   # [k, q]

    scale = 1.0 / math.sqrt(HD)
    # rope tables [128, S]: row p uses freq index p//2; sin sign folded:
    # even rows -sin (o_even = x_e*c - x_o*s), odd rows +sin.
    cexp = np.repeat(fc.T, 2, axis=0)                      # [128, S]
    sexp = np.repeat(fs.T, 2, axis=0)
    sign = np.where((np.arange(P) % 2) == 0, -1.0, 1.0)[:, None].astype(np.float32)
    cq_t = np.ascontiguousarray(cexp * scale)
    sq_t = np.ascontiguousarray(sexp * sign * scale)
    ck_t = np.ascontiguousarray(cexp)
    sk_t = np.ascontiguousarray(sexp * sign)

    in_maps = []
    for i in range(NCORES):
        e0 = i * M
        in_maps.append({
            "xT": xT,
            "wqT": np.ascontiguousarray(wq[e0:e0 + M, :].T),
            "wkT": np.ascontiguousarray(wk[e0:e0 + M, :].T),
            "wvT": np.ascontiguousarray(wv[e0:e0 + M, :].T),
            "woT": woT,
            "cq": cq_t,
            "sq": sq_t,
            "ck": ck_t,
            "sk": sk_t,
            "maskT": maskT,
        })
    return in_maps


def _run(inputs, trace=False, **kw):
    nc = _build()
    in_maps = _prep_in_maps(
        inputs["x"], inputs["wq"], inputs["wk"], inputs["wv"], inputs["wo"],
        inputs["freqs_cos"], inputs["freqs_sin"], inputs["mask"],
    )
    res = run_bass_kernel_spmd(nc, in_maps, list(range(NCORES)), trace=trace, **kw)
    shards = [res.results[j]["out"] for j in range(NCORES)]
    full = np.concatenate(shards, axis=0).reshape(B, S, D)
    return full, res


def kernel(**inputs):
    full, _ = _run(inputs, trace=False)
    return full


# revision 8
# speedup vs baseline: 1.1331x; 1.1331x over previous
"""Distributed Trainium2 (Bass/Tile) kernel for a llama-style attention layer.

Problem: B=4, S=1024, D=4096, H=32 heads, HD=128; fp32 in/out; start_pos=0,
cache starts zeroed (the cache update is a pure overwrite, so the layer is
exactly: QKV proj -> RoPE(q,k) -> softmax((q k^T)/sqrt(HD) + mask) v -> Wo).

Sharding (8 NeuronCores; tensor-parallel over heads, then all-to-all to
row-parallel for the output projection):
  phase 1: each core owns 4 heads; computes Q.T/K.T [hd, r] for its heads
           over all r = b*S + s rows (wq/wk column shards resident in SBUF,
           x streamed), applies RoPE in [hd(partition), r] layout via a
           partition pair-swap (stream_shuffle) with host-precomputed
           cos/sin tables (score 1/sqrt(HD) folded into the Q tables),
           spills Q.T/K.T to DRAM scratch.
  phase 2: per batch: V pass (V [s, m], wv resident, x streamed again), then
           attention per head computed transposed (S.T[k,q] = K @ Q.T) so no
           on-chip transposes are needed; softmax row-sums broadcast via an
           all-ones matmul; PV as out.T[hd, q] with N=512 matmuls; normalized
           out.T tiles land in the all-to-all input buffer.
  phase 3: one AllToAll (8 MB/core) turns head-sharding into row-sharding;
           each core computes its 512 rows of the final projection against
           the full (streamed) wo; host concatenates the row shards.

All matmuls run in float32r (fp32 with 11 explicit mantissa bits; full PE
rate at N=512). Matrix inputs are pre-rounded to the f32r grid on the host
(round-to-nearest-even dropping 12 mantissa bits — verified bit-exact vs the
hardware casting DMA), so every load is a plain full-rate HWDGE DMA.

If the mask is exactly causal (upper triangle <= -1e8, rest 0), a variant
that skips fully-masked score tiles is used (25% fewer attention matmuls,
half the mask adds).
"""

import math

import numpy as np

import concourse.bass as bass
import concourse.mybir as mybir
import concourse.tile as tile
from concourse import bacc
from concourse.bass_utils import run_bass_kernel_spmd

F32 = mybir.dt.float32
F32R = mybir.dt.float32r
BF16 = mybir.dt.bfloat16
AF = mybir.ActivationFunctionType

B, S, D, H = 4, 1024, 4096, 32
HD = 128
NCORES = 8
HL = H // NCORES   # 4 local heads
M = HL * HD        # 512 local head columns
R = B * S          # 4096 rows
RPC = R // NCORES  # 512 rows per core in phase 3
P = 128
DC = D // P        # 32 contraction chunks

_BUILD_CACHE = {}


def _build(causal):
    key = ("causal" if causal else "general")
    if key in _BUILD_CACHE:
        return _BUILD_CACHE[key]
    nc = bacc.Bacc(None, target_bir_lowering=False)

    xT = nc.declare_dram_parameter("xT", [D, R], F32, isOutput=False)
    wqT = nc.declare_dram_parameter("wqT", [D, M], F32, isOutput=False)
    wkT = nc.declare_dram_parameter("wkT", [D, M], F32, isOutput=False)
    wvT = nc.declare_dram_parameter("wvT", [D, M], F32, isOutput=False)
    woT = nc.declare_dram_parameter("woT", [D, D], F32, isOutput=False)
    cq = nc.declare_dram_parameter("cq", [P, S], F32, isOutput=False)
    sq = nc.declare_dram_parameter("sq", [P, S], F32, isOutput=False)
    ck = nc.declare_dram_parameter("ck", [P, S], F32, isOutput=False)
    sk = nc.declare_dram_parameter("sk", [P, S], F32, isOutput=False)
    maskT = nc.declare_dram_parameter("maskT", [S, S], F32, isOutput=False)
    out = nc.declare_dram_parameter("out", [RPC, D], F32, isOutput=True)

    pairmask = [p ^ 1 for p in range(32)]

    def n_kt(qh):
        # number of contributing k-tiles for this q-half
        return 4 * qh + 4 if causal else 8

    def crossing(kt, qh):
        # does this (k-tile, q-half) intersect the causal diagonal block?
        return (kt - 4 * qh) in (0, 1, 2, 3)

    with tile.TileContext(nc) as tc:
        with (
            tc.tile_pool(name="const", bufs=1) as cpool,
            tc.tile_pool(name="dram", bufs=1, space="DRAM") as dram,
        ):
            # DRAM scratch: QK spill [2(q/k), B, HL, 128, S] and a2a buffers
            qkd = dram.tile([2, B, HL, P, S], F32R, tag="qkd")
            a2a_in = dram.tile([R, RPC], F32, tag="a2a_in")
            a2a_out = dram.tile([R, RPC], F32, tag="a2a_out")

            ones_f = cpool.tile([P, P], F32, tag="ones_f")
            nc.vector.memset(ones_f[:], 1.0)
            ones128 = cpool.tile([P, P], F32R, tag="ones128")
            nc.vector.tensor_copy(ones128[:], ones_f[:])

            # ---------------- phase 1: Q.T / K.T + rope -> spill ----------
            with (
                tc.tile_pool(name="p1w", bufs=1) as wpool,
                tc.tile_pool(name="p1rope", bufs=1) as rpool,
                tc.tile_pool(name="p1x", bufs=6) as xpool,
                tc.tile_pool(name="p1s", bufs=2) as spool,
                tc.tile_pool(name="p1ps", bufs=8, space="PSUM") as ps1,
            ):
                cq_sb = rpool.tile([P, S], F32, tag="cq")
                sq_sb = rpool.tile([P, S], F32, tag="sq")
                ck_sb = rpool.tile([P, S], F32, tag="ck")
                sk_sb = rpool.tile([P, S], F32, tag="sk")
                nc.sync.dma_start(cq_sb[:], cq[:])
                nc.sync.dma_start(sq_sb[:], sq[:])
                nc.sync.dma_start(ck_sb[:], ck[:])
                nc.sync.dma_start(sk_sb[:], sk[:])
                wq_sb = []
                wk_sb = []
                for dc in range(DC):
                    tq = wpool.tile([P, M], F32R, tag=f"wq{dc}")
                    tk = wpool.tile([P, M], F32R, tag=f"wk{dc}")
                    nc.sync.dma_start(tq[:], wqT[dc * P:(dc + 1) * P, :].bitcast(F32R))
                    nc.sync.dma_start(tk[:], wkT[dc * P:(dc + 1) * P, :].bitcast(F32R))
                    wq_sb.append(tq)
                    wk_sb.append(tk)

                for b in range(B):
                    for rt in range(2):
                        r0 = b * S + rt * 512
                        sl = slice(rt * 512, rt * 512 + 512)
                        q_ps = [ps1.tile([P, 512], F32, tag="mm", name=f"qps{m}")
                                for m in range(HL)]
                        k_ps = [ps1.tile([P, 512], F32, tag="mm", name=f"kps{m}")
                                for m in range(HL)]
                        for dc in range(DC):
                            xt = xpool.tile([P, 512], F32R, tag="xt")
                            nc.sync.dma_start(
                                xt[:],
                                xT[dc * P:(dc + 1) * P, r0:r0 + 512].bitcast(F32R),
                            )
                            st = dc == 0
                            sp = dc == DC - 1
                            for m in range(HL):
                                nc.tensor.matmul(
                                    q_ps[m][:],
                                    wq_sb[dc][:, m * P:(m + 1) * P],
                                    xt[:],
                                    start=st, stop=sp,
                                )
                            for m in range(HL):
                                nc.tensor.matmul(
                                    k_ps[m][:],
                                    wk_sb[dc][:, m * P:(m + 1) * P],
                                    xt[:],
                                    start=st, stop=sp,
                                )
                        # rope: dst = src*c + pairswap(src)*s (sin sign folded)
                        for m in range(HL):
                            for qk, src, ctab, stab in (
                                (0, q_ps[m], cq_sb, sq_sb),
                                (1, k_ps[m], ck_sb, sk_sb),
                            ):
                                sh = spool.tile([P, 512], F32, tag="ropesh")
                                nc.vector.stream_shuffle(sh[:], src[:], pairmask)
                                t1 = spool.tile([P, 512], F32, tag="ropet1")
                                nc.vector.tensor_mul(t1[:], src[:], ctab[:, sl])
                                dst = spool.tile([P, 512], F32R, tag="ropedst")
                                nc.vector.tensor_mul(sh[:], sh[:], stab[:, sl])
                                nc.vector.tensor_add(dst[:], t1[:], sh[:])
                                nc.sync.dma_start(qkd[qk, b, m, :, sl], dst[:])

            # ---------------- phase 2: V pass + attention ----------------
            with (
                tc.tile_pool(name="p2w", bufs=1) as vwpool,
                tc.tile_pool(name="p2v", bufs=2) as vpool,
                tc.tile_pool(name="p2x", bufs=6) as xpool2,
                tc.tile_pool(name="p2qk", bufs=2) as qkpool,
                tc.tile_pool(name="p2ep", bufs=10) as eppool,
                tc.tile_pool(name="p2s", bufs=2) as apool,
                tc.tile_pool(name="p2m", bufs=1) as mpool,
                tc.tile_pool(name="p2ps", bufs=1, space="PSUM") as ps2,
            ):
                mk = {}
                for kt in range(8):
                    for qh in range(2):
                        if causal and not crossing(kt, qh):
                            continue
                        t = mpool.tile([P, 512], BF16, tag=f"mk{kt}_{qh}")
                        nc.gpsimd.dma_start(
                            t[:],
                            maskT[kt * P:(kt + 1) * P, qh * 512:(qh + 1) * 512],
                        )
                        mk[(kt, qh)] = t
                wv_sb = []
                for dc in range(DC):
                    tv = vwpool.tile([P, M], F32R, tag=f"wv{dc}")
                    nc.sync.dma_start(tv[:], wvT[dc * P:(dc + 1) * P, :].bitcast(F32R))
                    wv_sb.append(tv)

                for b in range(B):
                    V = [vpool.tile([P, M], F32R, tag=f"V{i}", name=f"V{i}")
                         for i in range(8)]
                    for rt in range(2):
                        r0 = b * S + rt * 512
                        v_ps = [
                            ps2.tile([P, M], F32, tag="vps", bufs=4, name=f"vps{i}")
                            for i in range(4)
                        ]
                        for dc in range(DC):
                            xt = xpool2.tile([P, 512], F32R, tag="xt")
                            nc.sync.dma_start(
                                xt[:],
                                xT[dc * P:(dc + 1) * P, r0:r0 + 512].bitcast(F32R),
                            )
                            st = dc == 0
                            sp = dc == DC - 1
                            for sub in range(4):
                                nc.tensor.matmul(
                                    v_ps[sub][:],
                                    xt[:, sub * P:(sub + 1) * P],
                                    wv_sb[dc][:],
                                    start=st, stop=sp,
                                )
                        for sub in range(4):
                            nc.scalar.copy(V[rt * 4 + sub][:], v_ps[sub][:])

                    for h in range(HL):
                        QT = qkpool.tile([P, S], F32R, tag="QT")
                        KT = qkpool.tile([P, S], F32R, tag="KT")
                        nc.sync.dma_start(QT[:], qkd[0, b, h, :, :])
                        nc.sync.dma_start(KT[:], qkd[1, b, h, :, :])
                        for qh in range(2):
                            qsl = slice(qh * 512, qh * 512 + 512)
                            nk = n_kt(qh)
                            ep = []
                            for kt in range(nk):
                                stile = ps2.tile([P, 512], F32, tag="st", bufs=2,
                                                 name="stile")
                                nc.tensor.matmul(
                                    stile[:],
                                    KT[:, kt * P:(kt + 1) * P],
                                    QT[:, qsl],
                                    start=True, stop=True,
                                )
                                if not causal or crossing(kt, qh):
                                    nc.vector.tensor_add(
                                        stile[:], stile[:], mk[(kt, qh)][:]
                                    )
                                e = eppool.tile([P, 512], F32R, tag="ep", name="e")
                                nc.scalar.activation(e[:], stile[:], AF.Exp)
                                ep.append(e)
                            # softmax denominator (pairwise tree)
                            def tree_add(tiles):
                                lvl = list(tiles)
                                tmpn = 0
                                while len(lvl) > 2:
                                    nxt = []
                                    for i in range(0, len(lvl) - 1, 2):
                                        t = apool.tile([P, 512], F32,
                                                       tag=f"tr{tmpn % 4}",
                                                       name="tadd")
                                        tmpn += 1
                                        nc.vector.tensor_add(
                                            t[:], lvl[i][:], lvl[i + 1][:]
                                        )
                                        nxt.append(t)
                                    if len(lvl) % 2:
                                        nxt.append(lvl[-1])
                                    lvl = nxt
                                return lvl
                            lvl = tree_add(ep)
                            accr = apool.tile([P, 512], F32R, tag="accr",
                                              name="accr")
                            if len(lvl) == 2:
                                nc.vector.tensor_add(accr[:], lvl[0][:], lvl[1][:])
                            else:
                                nc.vector.tensor_copy(accr[:], lvl[0][:])
                            # rowsum broadcast to all partitions in one matmul
                            bcp = ps2.tile([P, 512], F32, tag="small", bufs=1,
                                           name="bcp")
                            nc.tensor.matmul(
                                bcp[:], ones128[:], accr[:], start=True, stop=True
                            )
                            bc_sb = apool.tile([P, 512], F32, tag="bcsb",
                                               name="bc_sb")
                            nc.vector.reciprocal(bc_sb[:], bcp[:])
                            # PV: out.T [hd, q]
                            ot = ps2.tile([P, 512], F32, tag="ot", bufs=1,
                                          name="ot")
                            for kt in range(nk):
                                nc.tensor.matmul(
                                    ot[:],
                                    V[kt][:, h * HD:(h + 1) * HD],
                                    ep[kt][:],
                                    start=(kt == 0), stop=(kt == nk - 1),
                                )
                            otn = apool.tile([P, 512], F32R, tag="otn",
                                             name="otn")
                            nc.vector.tensor_mul(otn[:], ot[:], bc_sb[:])
                            j = b * 2 + qh
                            row0 = j * 512 + h * HD
                            nc.sync.dma_start(a2a_in[row0:row0 + HD, :].bitcast(F32R), otn[:])

            # ---------------- phase 3: all-to-all + output proj -----------
            nc.gpsimd.collective_compute(
                "AllToAll",
                mybir.AluOpType.bypass,
                replica_groups=[list(range(NCORES))],
                ins=[a2a_in.opt()],
                outs=[a2a_out.opt()],
            )
            with (
                tc.tile_pool(name="p3a", bufs=1) as fpool,
                tc.tile_pool(name="p3w", bufs=6) as wspool,
                tc.tile_pool(name="p3o", bufs=4) as fopool,
                tc.tile_pool(name="p3ps", bufs=8, space="PSUM") as ps3,
            ):
                a2a_sb = []
                for ec in range(DC):
                    t = fpool.tile([P, RPC], F32R, tag=f"a2a{ec}")
                    nc.sync.dma_start(t[:], a2a_out[ec * P:(ec + 1) * P, :].bitcast(F32R))
                    a2a_sb.append(t)
                for dt_ in range(8):
                    f_ps = [ps3.tile([P, 512], F32, tag="fps", name=f"fps{i}")
                            for i in range(4)]
                    for ec in range(DC):
                        wo_t = wspool.tile([P, 512], F32R, tag="wot")
                        nc.sync.dma_start(
                            wo_t[:],
                            woT[ec * P:(ec + 1) * P,
                                dt_ * 512:(dt_ + 1) * 512].bitcast(F32R),
                        )
                        st = ec == 0
                        sp = ec == DC - 1
                        for qs in range(4):
                            nc.tensor.matmul(
                                f_ps[qs][:],
                                a2a_sb[ec][:, qs * P:(qs + 1) * P],
                                wo_t[:],
                                start=st, stop=sp,
                            )
                    for qs in range(4):
                        o_sb = fopool.tile([P, 512], F32, tag="osb")
                        nc.scalar.copy(o_sb[:], f_ps[qs][:])
                        nc.sync.dma_start(
                            out[qs * P:(qs + 1) * P, dt_ * 512:(dt_ + 1) * 512],
                            o_sb[:],
                        )

    nc.compile()
    _BUILD_CACHE[key] = nc
    return nc


def _round_f32r(a):
    """Round fp32 -> float32r grid (RNE dropping 12 mantissa bits).

    Bit-exact vs the hardware f32->f32r casting DMA (verified empirically).
    """
    b = np.ascontiguousarray(a, dtype=np.float32).view(np.uint32).astype(np.uint64)
    low = np.uint64(1 << 12)
    half = np.uint64(1 << 11)
    r = b + half
    tie = (b & (low - np.uint64(1))) == half
    r_tie = b + np.where((b >> np.uint64(12)) & np.uint64(1) == np.uint64(1),
                         half, np.uint64(0))
    r = np.where(tie, r_tie, r) & ~(low - np.uint64(1))
    return r.astype(np.uint32).view(np.float32).reshape(a.shape)


def _is_causal(mask2d):
    iu = np.triu_indices(S, k=1)
    if not np.all(mask2d[iu] <= -1e8):
        return False
    il = np.tril_indices(S, k=0)
    return np.all(mask2d[il] == 0.0)


def _prep_in_maps(x, wq, wk, wv, wo, freqs_cos, freqs_sin, mask):
    x = np.asarray(x, dtype=np.float32)
    wq = np.asarray(wq, dtype=np.float32)
    wk = np.asarray(wk, dtype=np.float32)
    wv = np.asarray(wv, dtype=np.float32)
    wo = np.asarray(wo, dtype=np.float32)
    fc = np.asarray(freqs_cos, dtype=np.float32)  # [S, HD//2]
    fs = np.asarray(freqs_sin, dtype=np.float32)
    mask2d = np.asarray(mask, dtype=np.float32).reshape(S, S)

    xT = _round_f32r(np.ascontiguousarray(x.reshape(R, D).T))   # [d, r]
    woT = _round_f32r(np.ascontiguousarray(wo.T))                # [e, dout]
    maskT = np.ascontiguousarray(mask2d.T)

    scale = 1.0 / math.sqrt(HD)
    # rope tables [128, S]: row p uses freq index p//2; sin sign folded:
    # even rows -sin (o_even = x_e*c - x_o*s), odd rows +sin.
    cexp = np.repeat(fc.T, 2, axis=0)                            # [128, S]
    sexp = np.repeat(fs.T, 2, axis=0)
    sign = np.where((np.arange(P) % 2) == 0, -1.0, 1.0)[:, None].astype(np.float32)
    cq_t = np.ascontiguousarray(cexp * scale)
    sq_t = np.ascontiguousarray(sexp * sign * scale)
    ck_t = np.ascontiguousarray(cexp)
    sk_t = np.ascontiguousarray(sexp * sign)

    in_maps = []
    for i in range(NCORES):
        e0 = i * M
        in_maps.append({
            "xT": xT,
            "wqT": _round_f32r(np.ascontiguousarray(wq[e0:e0 + M, :].T)),
            "wkT": _round_f32r(np.ascontiguousarray(wk[e0:e0 + M, :].T)),
            "wvT": _round_f32r(np.ascontiguousarray(wv[e0:e0 + M, :].T)),
            "woT": woT,
            "cq": cq_t,
            "sq": sq_t,
            "ck": ck_t,
            "sk": sk_t,
            "maskT": maskT,
        })
    return in_maps, _is_causal(mask2d)


def _run(inputs, trace=False, **kw):
    in_maps, causal = _prep_in_maps(
        inputs["x"], inputs["wq"], inputs["wk"], inputs["wv"], inputs["wo"],
        inputs["freqs_cos"], inputs["freqs_sin"], inputs["mask"],
    )
    nc = _build(causal)
    res = run_bass_kernel_spmd(nc, in_maps, list(range(NCORES)), trace=trace, **kw)
    shards = [res.results[j]["out"] for j in range(NCORES)]
    full = np.concatenate(shards, axis=0).reshape(B, S, D)
    return full, res


def kernel(**inputs):
    full, _ = _run(inputs, trace=False)
    return full
